# revision 19
# baseline (speedup 1.0000x reference)
"""Trainium2 Bass kernel for nn_Blur: depthwise 4x4 FIR blur (upfirdn2d pad=(2,1)).

Full inputs in, full output out. Internally shards the 4096 (b,c) images
across 8 NeuronCores (pure data parallel, no collectives).

Device I/O is bf16 (tolerance is rel_err < 2e-2; halves HBM traffic, which
is the binding roofline).  The host pre-packs x into a gap layout ([H,
n*130+4]: 2 zero cols between images) so shifted moving-operand reads of
the matmuls see zero padding at image edges and every DMA is one large
per-partition-contiguous transfer.

Compute per core (512 images of [H=128, W=128]), two paths mixed per tile
so PE / Vector / Scalar all sit below the DMA stream rate:

 - prebox path (taps proportional to [1,3,3,1], the actual blur — uses
   [1,3,3,1] = [1,1]*[1,2,1]): Vector pre-convolves the RAW INPUT with
   [1,2,1] along W (two flat box-2 tensor_tensor adds at 2x DVE mode),
   then PE applies the tap-scaled H band matrix at W-shifts {-2,-1} as a
   2-pass accumulating matmul (contraction over the partition/H axis),
   then one Scalar PSUM->SBUF evacuation per chunk.  Vector only ever
   feeds the input side, so evacuation/output never waits on DVE.
 - 4-pass path: the conv factors into 4 banded matmuls (one per W tap)
   with the W-shift realized as a shifted moving-operand read.  Carries
   ~3 images/tile plus fallbacks (non-[1,3,3,1] or non-separable kernels).

Matmuls within a chunk share one stationary; all but the first set
InstMatmult.ldweights=False to skip the redundant PE array load (confirmed
on HW: matmul spacing drops from ~201ns to ~167ns for 392-col matmuls).

Schedule: input tiles ride the Sync HWDGE ring (first x tile is the very
first Sync instruction), bands ride the Scalar ring, output tiles are
issued per-tile from GpSimd (SWDGE) so the Scalar evacuation stream never
blocks on a previous tile's output (the last drain tiles alternate rings).
Tiles ramp 6,6,12 -> 24 and de-ramp 12,6,6(,rag) so both the pipeline fill
and the post-last-input drain are short.  xpool depth 4 paces input DMA to
compute so the output stream gets bandwidth mid-run.  PE warm-up matmuls
run on a memset tile (no DMA dependency) to open the HAM clock gate early.
"""

import os
import sys
from contextlib import ExitStack

for _p in ("/opt/trn_rl_repo", "/root/.axon_site/_ro/trn_rl_repo"):
    if os.path.isdir(_p) and _p not in sys.path:
        sys.path.append(_p)

import ml_dtypes
import numpy as np

import concourse.bass as bass  # noqa: F401  (engine types referenced via nc)
import concourse.tile as tile
from concourse import bacc, bass_utils, mybir

BF16 = np.dtype(ml_dtypes.bfloat16)

B, C, H, W = 16, 256, 128, 128
N_CORES = 8
GROUP = 3          # images per PSUM bank / matmul group
STRIDE = 130       # 2-col gap + 128 data cols per image in the packed layout
PAD0 = 2           # upfirdn2d pad before (both spatial dims)
TILE = 24          # images per DMA tile (must be multiple of GROUP)
QG = 2             # matmul groups (= PSUM banks / 512 cols) per chunk tile

_PROGRAM_CACHE: dict[object, object] = {}


def _band_matrices(kern: np.ndarray) -> np.ndarray:
    """bands[j][hi, ho] = wf[hi-ho+2, j], wf = flip(kern). Shape [4,128,128]."""
    wf = np.flip(np.asarray(kern, dtype=np.float64), (0, 1))
    bands = np.zeros((4, H, H), dtype=np.float64)
    ho = np.arange(H)
    for j in range(4):
        for i in range(4):
            d = i - PAD0            # hi - ho
            hi = ho + d
            m = (hi >= 0) & (hi < H)
            bands[j][hi[m], ho[m]] = wf[i, j]
    return np.ascontiguousarray(bands.astype(np.float32))


def _tiles(n_images: int):
    """Split n_images into DMA tiles of at most TILE images.

    The first tiles ramp up small so the first matmul can start as soon as
    a small DMA lands; the last tiles ramp DOWN (de-ramp) so the backlog
    of compute still in flight when the final input lands — and therefore
    the drain tail after it — is small.  Any ragged remainder goes last.
    """
    ramp = [6, 6, 12]
    deramp = [12, 6, 6]
    rag = n_images % TILE if n_images > 48 else 0
    out = []
    i = 0
    for r in ramp:
        if n_images - i > r:
            out.append((i, r))
            i += r
    budget = n_images - i - rag - sum(deramp)
    while budget >= TILE:
        out.append((i, TILE))
        i += TILE
        budget -= TILE
    for r in deramp + ([rag] if rag else []):
        if r and i + r <= n_images:
            out.append((i, r))
            i += r
    while i < n_images:  # fallback for tiny n_images
        n = min(TILE, n_images - i)
        out.append((i, n))
        i += n
    return out


def _off_plan(tiles, mode, off_target):
    """Per-tile offload image counts (multiples of GROUP).

    Ramp (non-full) tiles are fully offloaded (PE is HAM-cold early);
    full tiles alternate 15/12 to hit off_target; the final ragged tile
    stays all-PE (tiny, and avoids a sub-GROUP offload segment).
    """
    n_t = len(tiles)
    segs = [0] * n_t
    if mode is None:
        return segs
    if mode == "box":
        # prebox plan: full tiles run SEG_FULL images through the 2-pass
        # path (rest 4-pass); small ramp/de-ramp tiles keep a GROUP-sized
        # 4-pass share so PE has direct-from-input work while Vector runs
        # the tile's u-chain (keeps the PE HAM clock gate open).
        for ti, (_, tn) in enumerate(tiles):
            if tn == TILE:
                segs[ti] = min(tn, off_target)
            else:
                segs[ti] = (tn // GROUP) * GROUP
        return segs
    total = 0
    full_idx = []
    for ti, (_, tn) in enumerate(tiles):
        if tn == TILE:
            full_idx.append(ti)
        elif ti < 3 and tn % GROUP == 0:
            s = (tn // 2 // GROUP) * GROUP   # ramp tiles: half offloaded
            segs[ti] = s
            total += s
    want = max(0, off_target - total)
    n_full = len(full_idx)
    if n_full:
        base = min(TILE, 3 * (want // (3 * n_full)))
        extra = (want - base * n_full + 2) // 3   # tiles that get +3
        for k, ti in enumerate(full_idx):
            s = base + (3 if k < extra else 0)
            segs[ti] = min(TILE, max(0, s))
    return segs


def _groups(n_images: int):
    """Split a tile's images into matmul groups of at most GROUP, avoiding a
    trailing 1-image group (rebalance 3+1 -> 2+2)."""
    out = []
    i = 0
    while i < n_images:
        n = min(GROUP, n_images - i)
        out.append((i, n))
        i += n
    if len(out) >= 2 and out[-1][1] == 1:
        i0, n0 = out[-2]
        out[-2] = (i0, 2)
        out[-1] = (i0 + 2, 2)
    return out


def build_program(n_images: int, mode=None, taps=None, off_target=282,
                  xt_bufs: int = 4):
    """Build + compile the per-core Bass program for n_images [128,128] images.

    DRAM layout (host-prepared, bf16):
      x: [H, n_images*STRIDE + 2]  image k's column w at STRIDE*k + 2 + w,
         cols {STRIDE*k, STRIDE*k+1} and the trailing 2 are zeros.
      y: [H, n_images*W]           image k's column w at W*k + w.

    bands[0:4] are the fused HxW band matrices (4-pass path); bands[4] is
    the H-only band (offload path; for mode='box' it is pre-scaled by the
    W tap scale so the box chain needs no multiply).

    mode: None (all 4-pass) | 'box' (taps prop. to [1,3,3,1], 3-add chain)
          | 'sym' (symmetric taps, 5-op chain) | 'gen' (7-op chain).
    """
    if mode == "box" and off_target > TILE:
        off_target = 21          # SEG_FULL: prebox images per full tile
    nc = bacc.Bacc("TRN2", target_bir_lowering=False, debug=False)
    f32 = mybir.dt.float32
    bf16 = mybir.dt.bfloat16

    x_d = nc.dram_tensor("x", [H, n_images * STRIDE + 4], bf16, kind="ExternalInput")
    b_d = nc.dram_tensor("bands", [5, H, H], bf16, kind="ExternalInput")
    y_d = nc.dram_tensor("y", [H, n_images * W], bf16, kind="ExternalOutput")

    tiles = _tiles(n_images)
    segs = _off_plan(tiles, mode, off_target)

    with ExitStack() as ctx:
        tc = ctx.enter_context(tile.TileContext(nc))
        wpool = ctx.enter_context(tc.tile_pool(name="wpool", bufs=1))
        xpool = ctx.enter_context(tc.tile_pool(name="xpool", bufs=xt_bufs))
        opool = ctx.enter_context(tc.tile_pool(name="opool", bufs=5))
        tapool = ctx.enter_context(tc.tile_pool(name="tapool", bufs=3))
        wkpool = ctx.enter_context(tc.tile_pool(name="wkpool", bufs=6))
        tailpool = ctx.enter_context(tc.tile_pool(name="tailpool", bufs=3))
        ppool = ctx.enter_context(tc.tile_pool(name="ppool", bufs=4, space="PSUM"))

        # All DMA via the two HWDGE rings.  Input tiles ride the SP (sync)
        # ring; the band matrices + output tiles ride the ACT (scalar) ring,
        # which is idle early.  GpSimd (SWDGE) stays fully idle.
        xts: dict[int, object] = {}

        def emit_in_dma(ti, pool=None):
            if ti in xts or ti >= len(tiles):
                return
            i0, tn = tiles[ti]
            p = pool if pool is not None else xpool
            xt = p.tile([H, tn * STRIDE + 4], bf16, tag="xt", name="xt")
            nc.sync.dma_start(
                xt, x_d[:, i0 * STRIDE : i0 * STRIDE + tn * STRIDE + 4]
            )
            xts[ti] = xt

        # The last TAILN tiles' inputs are prefetched early into a dedicated
        # pool: the drain compute chains are then never input-gated, which
        # keeps the final output DMAs back-to-back (and the PE busy enough
        # that HAM doesn't re-throttle it mid-drain).
        TAILN = 3
        tail_start = max(0, len(tiles) - TAILN)

        # First x tile DMA is the very first Sync-ring instruction.
        emit_in_dma(0)

        wt = wpool.tile([H, 5 * H], bf16)
        nc.scalar.dma_start(
            wt.rearrange("p (j b) -> p j b", b=H), b_d.rearrange("j a b -> a j b")
        )

        # Warm up the PE HAM clock gate with dummy matmuls on a memset tile
        # (no DMA dependency, so warm-up starts as soon as the engines come
        # out of the framework preamble): real matmuls then hit 2.4 GHz
        # within ~1-2us instead of ~5.
        wsrc = wkpool.tile([H, 256], bf16, tag="wk", name="wk")
        nc.vector.memset(wsrc, 0)
        warm = ppool.tile([H, 512 * QG], f32, tag="pt", name="pt")
        for _ in range(18):
            nc.tensor.matmul(
                warm[:, 0:256], wsrc[:, 0:H], wsrc[:, 0:256], start=True, stop=True
            )

        for ti in range(1, min(3, len(tiles))):
            if ti < tail_start:
                emit_in_dma(ti)
        # Tail inputs go out right here, at the head of the Sync ring's
        # FIFO: the early phase is input-only with spare bandwidth, whereas
        # a mid-run insert would delay the paced input stream behind it.
        for tj in range(tail_start, len(tiles)):
            emit_in_dma(tj, pool=tailpool)

        def tA_view(tA, d, nseg):
            """[p, nseg, W] view of the gap-layout tile shifted by d cols."""
            span = nseg * STRIDE
            if d <= 0:
                sl = tA[:, PAD0 + d : PAD0 + d + span]
                lo = 0
            else:
                sl = tA[:, PAD0 : PAD0 + span]
                lo = d
            return sl.rearrange("p (k c) -> p k c", c=STRIDE)[:, :, lo : lo + W]

        # Per-tile output DMA.  In box mode it is issued from GpSimd (SWDGE,
        # otherwise idle) so the Scalar engine's strict-FIFO evacuation
        # stream is never blocked waiting on Vector's W-conv for a previous
        # tile — that serialization stalled PSUM drain and re-throttled the
        # PE HAM clock.  (Safe here: box-mode Vector only uses 1-port DVE
        # modes, so Q7 descriptor-gen doesn't contend for SBUF ports.)
        out_dma_eng = nc.gpsimd if mode == "box" else nc.scalar

        def emit_4pass(xt, ot, img0, n_img, drain=None):
            """4-pass fused HxW chunks for images [img0, img0+n_img) of the
            tile, reading xt directly (gap layout); evacs on Scalar."""
            nonlocal copy_idx
            gs = _groups(n_img)
            chunks = [gs[s : s + QG] for s in range(0, len(gs), QG)]
            for chunk in chunks:
                nq = len(chunk)
                pt = ppool.tile([H, 512 * nq], f32, tag="pt", name="pt")
                # j-outer order amortizes the 4 stationary (band) loads over
                # the whole chunk; j=2 (d=0) first for the full-width
                # has_written-clearing write.
                for idx, j in enumerate((2, 0, 1, 3)):
                    d = j - PAD0
                    for q, (goff, n) in enumerate(chunk):
                        a = PAD0
                        b = STRIDE * n + PAD0 - (PAD0 if d > 0 else 0)
                        base = (img0 + goff) * STRIDE
                        mm = nc.tensor.matmul(
                            pt[:, 512 * q + a : 512 * q + b],
                            wt[:, H * j : H * (j + 1)],
                            xt[:, base + a + d : base + b + d],
                            start=(idx == 0),
                            stop=(idx == 3),
                        )
                        if q > 0:  # same band as q==0: reuse the stationary
                            mm.ins.ldweights = False
                emit_evac(pt, chunk, ot, img0, drain)

        def emit_prebox(xt, ot, n_img, drain=None):
            """2-pass path for images [0, n_img): Vector pre-convolves the
            raw input with [1,2,1] along W (two flat box-2 adds, 2x DVE
            mode), then PE applies the scaled H band at W-shifts {-2,-1}:
              u1[c] = x[c] + x[c+1]
              u2[c] = u1[c] + u1[c+1]            ( = x *w [1,2,1] )
              psum[:, c] = Bh_s^T (u2[:, c-2] + u2[:, c-1])   ( full blur )
            Vector feeds the INPUT side only — no Scalar->Vector->output
            chain — so evacuation/output never waits on DVE progress."""
            nonlocal copy_idx
            L = n_img * STRIDE + 4
            wk1 = wkpool.tile([H, L - 1], bf16, tag="wk", name="wk")
            wk2 = wkpool.tile([H, L - 2], bf16, tag="wk", name="wk")
            nc.vector.tensor_add(wk1, xt[:, 0 : L - 1], xt[:, 1:L])
            nc.vector.tensor_add(wk2, wk1[:, 0 : L - 2], wk1[:, 1 : L - 1])
            gs = _groups(n_img)
            chunks = [gs[s : s + QG] for s in range(0, len(gs), QG)]
            for chunk in chunks:
                nq = len(chunk)
                pt = ppool.tile([H, 512 * nq], f32, tag="pt", name="pt")
                first = True
                for idx, dd in enumerate((-2, -1)):
                    for q, (goff, n) in enumerate(chunk):
                        a = PAD0
                        b = STRIDE * n + PAD0
                        base = goff * STRIDE
                        mm = nc.tensor.matmul(
                            pt[:, 512 * q + a : 512 * q + b],
                            wt[:, 4 * H : 5 * H],
                            wk2[:, base + a + dd : base + b + dd],
                            start=(idx == 0),
                            stop=(idx == 1),
                        )
                        # all matmuls in this chunk share one stationary
                        # (the scaled H band): only the first loads the PE
                        # array; the rest reuse it (skip LDWEIGHTS).
                        if not first:
                            mm.ins.ldweights = False
                        first = False
                emit_evac(pt, chunk, ot, 0, drain)

        def emit_evac(pt, chunk, ot, img0, drain=None):
            """PSUM -> SBUF evacuation (fp32 -> bf16, gap strip).  One
            strided copy per chunk when the chunk is uniform (all groups
            GROUP-sized); per-group copies otherwise (ragged tail).

            drain=(tile_i0,) streams this chunk's images straight out after
            the copy (small per-chunk DMAs, alternating rings): in the
            kernel's final tiles this overlaps the last evacuations with
            the output drain instead of waiting for the whole tile."""
            nonlocal copy_idx, dma_alt
            nq = len(chunk)
            uniform = all(n == GROUP for _, n in chunk)
            if mode is not None:
                # All evacs on Scalar: a Vector-assigned evac would sit in
                # Vector's FIFO ahead of the next tile's input-side work.
                eng = (nc.scalar, nc.scalar)
            else:
                eng = (nc.vector, nc.scalar)
            if uniform:
                psrc = (
                    pt.rearrange("p (q c) -> p q c", c=512)[:, :, : GROUP * STRIDE]
                    .rearrange("p q (k c) -> p q k c", c=STRIDE)[
                        :, :, :, PAD0 : PAD0 + W
                    ]
                )
                odst = ot[
                    :,
                    (img0 + chunk[0][0]) * W : (img0 + chunk[-1][0] + GROUP) * W,
                ].rearrange("p (q k c) -> p q k c", q=nq, c=W)
                e = eng[copy_idx % 2]
                if e is nc.vector:
                    e.tensor_copy(odst, psrc)
                else:
                    e.copy(odst, psrc)
                copy_idx += 1
            else:
                for q, (goff, n) in enumerate(chunk):
                    psrc = pt[:, 512 * q : 512 * q + STRIDE * n].rearrange(
                        "p (k c) -> p k c", c=STRIDE
                    )[:, :, PAD0 : PAD0 + W]
                    odst = ot[
                        :, (img0 + goff) * W : (img0 + goff + n) * W
                    ].rearrange("p (k c) -> p k c", c=W)
                    e = eng[copy_idx % 2]
                    if e is nc.vector:
                        e.tensor_copy(odst, psrc)
                    else:
                        e.copy(odst, psrc)
                    copy_idx += 1
            if drain is not None:
                tile_i0 = drain[0]
                lo = img0 + chunk[0][0]
                hi = img0 + chunk[-1][0] + chunk[-1][1]
                e_dma = nc.scalar if dma_alt % 2 == 0 else nc.gpsimd
                dma_alt += 1
                e_dma.dma_start(
                    y_d[:, (tile_i0 + lo) * W : (tile_i0 + hi) * W],
                    ot[:, lo * W : hi * W],
                )

        copy_idx = 0
        dma_alt = 0
        n_t = len(tiles)
        for ti, (i0, tn) in enumerate(tiles):
            if ti + 3 < tail_start:
                emit_in_dma(ti + 3)
            xt = xts.pop(ti)

            seg = segs[ti]
            n_norm = tn - seg
            ot = opool.tile([H, tn * W], bf16, tag="ot", name="ot")

            if mode == "box":
                # prebox portion is images [0, seg); 4-pass the rest.  The
                # u-chain goes first (Vector, input side); the 4-pass chunks
                # (independent of it) keep PE busy while it runs.
                if n_norm:
                    emit_4pass(xt, ot, seg, n_norm)
                if seg:
                    emit_prebox(xt, ot, seg)
                # Drain tiles alternate output rings (SWDGE + ACT HWDGE):
                # two queues keep the SDMA engines fed in the output-only
                # tail, and Scalar is idle by then anyway.
                if ti >= n_t - 6:
                    eng_out = nc.scalar if (n_t - ti) % 2 == 0 else nc.gpsimd
                else:
                    eng_out = nc.gpsimd
                eng_out.dma_start(y_d[:, i0 * W : (i0 + tn) * W], ot)
                continue

            # --- non-box modes: original structure (offload seg last, via
            # tA intermediate; 4-pass first) ---
            if seg:
                segc0 = n_norm * STRIDE
                span = seg * STRIDE + 2
                tA = tapool.tile([H, span], bf16, tag="ta", name="ta")
                c0 = 0
                while c0 < span:
                    cw = min(512 * QG, span - c0)
                    pt = ppool.tile([H, 512 * QG], f32, tag="pt", name="pt")
                    s = 0
                    while s < cw:
                        w_ = min(512, cw - s)
                        nc.tensor.matmul(
                            pt[:, s : s + w_],
                            wt[:, 4 * H : 5 * H],
                            xt[:, segc0 + c0 + s : segc0 + c0 + s + w_],
                            start=True,
                            stop=True,
                        )
                        s += w_
                    nc.scalar.copy(tA[:, c0 : c0 + cw], pt[:, 0:cw])
                    c0 += cw

                e = nc.vector
                otv = ot[:, n_norm * W : tn * W].rearrange("p (k c) -> p k c", c=W)
                if True:
                    v = [tA_view(tA, d, seg) for d in (-2, -1, 0, 1)]
                    wk1 = wkpool.tile([H, seg * W], bf16, tag="wk", name="wk")
                    wk2 = wkpool.tile([H, seg * W], bf16, tag="wk", name="wk")
                    wk3 = wkpool.tile([H, seg * W], bf16, tag="wk", name="wk")
                    w1v = wk1.rearrange("p (k c) -> p k c", c=W)
                    w2v = wk2.rearrange("p (k c) -> p k c", c=W)
                    w3v = wk3.rearrange("p (k c) -> p k c", c=W)
                    if mode == "sym":
                        e.tensor_add(w1v, v[0], v[3])
                        e.tensor_add(w2v, v[1], v[2])
                        e.tensor_scalar_mul(w3v, w1v, taps[0])
                        e.tensor_scalar_mul(w1v, w2v, taps[1])
                        e.tensor_add(otv, w3v, w1v)
                    else:
                        e.tensor_scalar_mul(w1v, v[0], taps[0])
                        e.tensor_scalar_mul(w2v, v[1], taps[1])
                        e.tensor_add(w3v, w1v, w2v)
                        e.tensor_scalar_mul(w1v, v[2], taps[2])
                        e.tensor_add(w2v, w3v, w1v)
                        e.tensor_scalar_mul(w1v, v[3], taps[3])
                        e.tensor_add(otv, w2v, w1v)

            gs = _groups(n_norm)
            chunks = [gs[s : s + QG] for s in range(0, len(gs), QG)]

            for chunk in chunks:
                nq = len(chunk)
                pt = ppool.tile([H, 512 * nq], f32, tag="pt", name="pt")
                # j-outer order amortizes the 4 stationary (band) loads over
                # the whole chunk; j=2 (d=0) first for the full-width
                # has_written-clearing write.
                for idx, j in enumerate((2, 0, 1, 3)):
                    d = j - PAD0
                    for q, (goff, n) in enumerate(chunk):
                        a = PAD0
                        b = STRIDE * n + PAD0 - (PAD0 if d > 0 else 0)
                        base = goff * STRIDE
                        nc.tensor.matmul(
                            pt[:, 512 * q + a : 512 * q + b],
                            wt[:, H * j : H * (j + 1)],
                            xt[:, base + a + d : base + b + d],
                            start=(idx == 0),
                            stop=(idx == 3),
                        )

                # PSUM -> SBUF evacuation (fp32 -> bf16).  One strided copy
                # per chunk when the chunk is uniform (all groups GROUP-sized);
                # per-group copies otherwise (ragged tail).  Scalar carries
                # the evacuations; Vector relieves it on every 8th chunk.
                uniform = all(n == GROUP for _, n in chunk)
                if mode is not None:
                    e_pick = nc.vector if copy_idx % 8 == 7 else nc.scalar
                    eng = (e_pick, e_pick)
                else:
                    eng = (nc.vector, nc.scalar)
                if uniform:
                    psrc = (
                        pt.rearrange("p (q c) -> p q c", c=512)[
                            :, :, : GROUP * STRIDE
                        ]
                        .rearrange("p q (k c) -> p q k c", c=STRIDE)[
                            :, :, :, PAD0 : PAD0 + W
                        ]
                    )
                    odst = ot[
                        :, chunk[0][0] * W : (chunk[-1][0] + GROUP) * W
                    ].rearrange("p (q k c) -> p q k c", q=nq, c=W)
                    e = eng[copy_idx % 2]
                    if e is nc.vector:
                        e.tensor_copy(odst, psrc)
                    else:
                        e.copy(odst, psrc)
                    copy_idx += 1
                else:
                    for q, (goff, n) in enumerate(chunk):
                        psrc = pt[:, 512 * q : 512 * q + STRIDE * n].rearrange(
                            "p (k c) -> p k c", c=STRIDE
                        )[:, :, PAD0 : PAD0 + W]
                        odst = ot[:, goff * W : (goff + n) * W].rearrange(
                            "p (k c) -> p k c", c=W
                        )
                        e = eng[copy_idx % 2]
                        if e is nc.vector:
                            e.tensor_copy(odst, psrc)
                        else:
                            e.copy(odst, psrc)
                        copy_idx += 1

            out_dma_eng.dma_start(y_d[:, i0 * W : (i0 + tn) * W], ot)

    nc.compile()
    return nc


def _get_program(n_images: int, mode=None, taps=None, off_target=282):
    key = (n_images, mode, taps, off_target)
    if key not in _PROGRAM_CACHE:
        _PROGRAM_CACHE[key] = build_program(
            n_images, mode=mode, taps=taps, off_target=off_target
        )
    return _PROGRAM_CACHE[key]


def _separable(kern: np.ndarray):
    """Return (bands5_f32, mode, taps).

    mode None: not rank-1 (all-PE 4-pass).  mode 'box': W taps proportional
    to [1,3,3,1]; the scale is folded into bands5[4] and taps is None.
    mode 'sym'/'gen': rank-1 with symmetric/general taps (5/7-op W-conv).
    """
    K = np.asarray(kern, dtype=np.float64)
    bands5 = np.zeros((5, H, H), dtype=np.float32)
    bands5[0:4] = _band_matrices(kern)
    U, S, Vt = np.linalg.svd(K)
    if S[1] > 1e-6 * max(S[0], 1e-30):
        return bands5, None, None
    a = U[:, 0] * np.sqrt(S[0])
    b = Vt[0, :] * np.sqrt(S[0])
    af = a[::-1]  # flipped H factor
    bfl = b[::-1]  # flipped W factor -> the 4 free-dim taps

    ref = np.array([1.0, 3.0, 3.0, 1.0])
    s = bfl[0]
    if abs(s) > 1e-30 and np.allclose(bfl, s * ref, rtol=1e-5, atol=0):
        mode, taps, af_eff = "box", None, af * s
    else:
        mode = "sym" if abs(bfl[0] - bfl[3]) <= 1e-7 * max(
            abs(bfl[0]), abs(bfl[3])
        ) and abs(bfl[1] - bfl[2]) <= 1e-7 * max(abs(bfl[1]), abs(bfl[2])) else "gen"
        taps, af_eff = tuple(float(np.float32(v)) for v in bfl), af

    ho = np.arange(H)
    Bh = np.zeros((H, H), dtype=np.float64)
    for i in range(4):
        hi = ho + (i - PAD0)
        m = (hi >= 0) & (hi < H)
        Bh[hi[m], ho[m]] = af_eff[i]
    bands5[4] = Bh.astype(np.float32)
    return bands5, mode, taps


def _pack_input(xc_bf16: np.ndarray) -> np.ndarray:
    """[n, H, W] bf16 -> [H, n*STRIDE + 4] bf16 gap layout."""
    n = xc_bf16.shape[0]
    arr = np.zeros((H, n * STRIDE + 4), dtype=BF16)
    v = np.lib.stride_tricks.as_strided(
        arr,
        shape=(H, n, STRIDE),
        strides=(arr.strides[0], STRIDE * arr.itemsize, arr.itemsize),
    )
    v[:, :, PAD0:] = xc_bf16.transpose(1, 0, 2)
    return arr


def kernel(x: np.ndarray, kernel: np.ndarray, _trace: bool = False):
    x = np.ascontiguousarray(x, dtype=np.float32)
    assert x.shape == (B, C, H, W), x.shape
    bands5, mode, taps = _separable(kernel)
    bands_bf = bands5.astype(BF16)

    n_total = B * C
    n_per_core = n_total // N_CORES
    xb = x.reshape(n_total, H, W).astype(BF16)

    nc = _get_program(n_per_core, mode, taps)
    in_maps = [
        {
            "x": _pack_input(xb[c * n_per_core : (c + 1) * n_per_core]),
            "bands": bands_bf,
        }
        for c in range(N_CORES)
    ]
    res = bass_utils.run_bass_kernel_spmd(
        nc, in_maps, core_ids=list(range(N_CORES)), trace=_trace
    )
    y = np.empty((n_total, H, W), dtype=np.float32)
    for c, r in enumerate(res.results):
        yc = np.asarray(r["y"]).reshape(H, n_per_core, W)
        y[c * n_per_core : (c + 1) * n_per_core] = yc.transpose(1, 0, 2).astype(
            np.float32
        )
    y = y.reshape(B, C, H, W)
    if _trace:
        return y, res
    return y


# revision 20
# speedup vs baseline: 1.0621x; 1.0621x over previous
"""Trainium2 Bass kernel for nn_Blur: depthwise 4x4 FIR blur (upfirdn2d pad=(2,1)).

Full inputs in, full output out. Internally shards the 4096 (b,c) images
across 8 NeuronCores (pure data parallel, no collectives).

Device I/O is bf16 (tolerance is rel_err < 2e-2; halves HBM traffic, which
is the binding roofline).  The host pre-packs x into a gap layout ([H,
n*130+4]: 2 zero cols between images) so shifted moving-operand reads of
the matmuls see zero padding at image edges and every DMA is one large
per-partition-contiguous transfer.

Compute per core (512 images of [H=128, W=128]), two paths mixed per tile
so PE / Vector / Scalar all sit below the DMA stream rate:

 - prebox path (taps proportional to [1,3,3,1], the actual blur — uses
   [1,3,3,1] = [1,1]*[1,2,1]): Vector pre-convolves the RAW INPUT with
   [1,2,1] along W (two flat box-2 tensor_tensor adds at 2x DVE mode),
   then PE applies the tap-scaled H band matrix at W-shifts {-2,-1} as a
   2-pass accumulating matmul (contraction over the partition/H axis),
   then one Scalar PSUM->SBUF evacuation per chunk.  Vector only ever
   feeds the input side, so evacuation/output never waits on DVE.
 - 4-pass path: the conv factors into 4 banded matmuls (one per W tap)
   with the W-shift realized as a shifted moving-operand read.  Carries
   ~3 images/tile plus fallbacks (non-[1,3,3,1] or non-separable kernels).

Matmuls within a chunk share one stationary; all but the first set
InstMatmult.ldweights=False to skip the redundant PE array load (confirmed
on HW: matmul spacing drops from ~201ns to ~167ns for 392-col matmuls).

Schedule: input tiles ride the Sync HWDGE ring (first x tile is the very
first Sync instruction), bands ride the Scalar ring, output tiles are
issued per-tile from GpSimd (SWDGE) so the Scalar evacuation stream never
blocks on a previous tile's output (the last drain tiles alternate rings).
Tiles ramp 6,6,12 -> 24 and de-ramp 12,6,6(,rag) so both the pipeline fill
and the post-last-input drain are short.  xpool depth 4 paces input DMA to
compute so the output stream gets bandwidth mid-run.  PE warm-up matmuls
run on a memset tile (no DMA dependency) to open the HAM clock gate early.
"""

import os
import sys
from contextlib import ExitStack

for _p in ("/opt/trn_rl_repo", "/root/.axon_site/_ro/trn_rl_repo"):
    if os.path.isdir(_p) and _p not in sys.path:
        sys.path.append(_p)

import ml_dtypes
import numpy as np

import concourse.bass as bass  # noqa: F401  (engine types referenced via nc)
import concourse.tile as tile
from concourse import bacc, bass_utils, mybir

BF16 = np.dtype(ml_dtypes.bfloat16)

B, C, H, W = 16, 256, 128, 128
N_CORES = 8
GROUP = 3          # images per PSUM bank / matmul group
STRIDE = 130       # 2-col gap + 128 data cols per image in the packed layout
PAD0 = 2           # upfirdn2d pad before (both spatial dims)
TILE = 24          # images per DMA tile (must be multiple of GROUP)
QG = 2             # matmul groups (= PSUM banks / 512 cols) per chunk tile

_PROGRAM_CACHE: dict[object, object] = {}


def _band_matrices(kern: np.ndarray) -> np.ndarray:
    """bands[j][hi, ho] = wf[hi-ho+2, j], wf = flip(kern). Shape [4,128,128]."""
    wf = np.flip(np.asarray(kern, dtype=np.float64), (0, 1))
    bands = np.zeros((4, H, H), dtype=np.float64)
    ho = np.arange(H)
    for j in range(4):
        for i in range(4):
            d = i - PAD0            # hi - ho
            hi = ho + d
            m = (hi >= 0) & (hi < H)
            bands[j][hi[m], ho[m]] = wf[i, j]
    return np.ascontiguousarray(bands.astype(np.float32))


def _tiles(n_images: int):
    """Split n_images into DMA tiles of at most TILE images.

    The first tiles ramp up small so the first matmul can start as soon as
    a small DMA lands; the last tiles ramp DOWN (de-ramp) so the backlog
    of compute still in flight when the final input lands — and therefore
    the drain tail after it — is small.  Any ragged remainder goes last.
    """
    ramp = [6, 6, 12]
    deramp = [12, 6, 6]
    rag = n_images % TILE if n_images > 48 else 0
    out = []
    i = 0
    for r in ramp:
        if n_images - i > r:
            out.append((i, r))
            i += r
    budget = n_images - i - rag - sum(deramp)
    while budget >= TILE:
        out.append((i, TILE))
        i += TILE
        budget -= TILE
    for r in deramp + ([rag] if rag else []):
        if r and i + r <= n_images:
            out.append((i, r))
            i += r
    while i < n_images:  # fallback for tiny n_images
        n = min(TILE, n_images - i)
        out.append((i, n))
        i += n
    return out


def _off_plan(tiles, mode, off_target):
    """Per-tile offload image counts (multiples of GROUP).

    Ramp (non-full) tiles are fully offloaded (PE is HAM-cold early);
    full tiles alternate 15/12 to hit off_target; the final ragged tile
    stays all-PE (tiny, and avoids a sub-GROUP offload segment).
    """
    n_t = len(tiles)
    segs = [0] * n_t
    if mode is None:
        return segs
    if mode == "box":
        # prebox plan: full tiles run SEG_FULL images through the 2-pass
        # path (rest 4-pass); small ramp/de-ramp tiles keep a GROUP-sized
        # 4-pass share so PE has direct-from-input work while Vector runs
        # the tile's u-chain (keeps the PE HAM clock gate open).
        for ti, (_, tn) in enumerate(tiles):
            if tn == TILE:
                segs[ti] = min(tn, off_target)
            else:
                segs[ti] = (tn // GROUP) * GROUP
        return segs
    total = 0
    full_idx = []
    for ti, (_, tn) in enumerate(tiles):
        if tn == TILE:
            full_idx.append(ti)
        elif ti < 3 and tn % GROUP == 0:
            s = (tn // 2 // GROUP) * GROUP   # ramp tiles: half offloaded
            segs[ti] = s
            total += s
    want = max(0, off_target - total)
    n_full = len(full_idx)
    if n_full:
        base = min(TILE, 3 * (want // (3 * n_full)))
        extra = (want - base * n_full + 2) // 3   # tiles that get +3
        for k, ti in enumerate(full_idx):
            s = base + (3 if k < extra else 0)
            segs[ti] = min(TILE, max(0, s))
    return segs


def _groups(n_images: int):
    """Split a tile's images into matmul groups of at most GROUP, avoiding a
    trailing 1-image group (rebalance 3+1 -> 2+2)."""
    out = []
    i = 0
    while i < n_images:
        n = min(GROUP, n_images - i)
        out.append((i, n))
        i += n
    if len(out) >= 2 and out[-1][1] == 1:
        i0, n0 = out[-2]
        out[-2] = (i0, 2)
        out[-1] = (i0 + 2, 2)
    return out


def build_program(n_images: int, mode=None, taps=None, off_target=282,
                  xt_bufs: int = 4):
    """Build + compile the per-core Bass program for n_images [128,128] images.

    DRAM layout (host-prepared, bf16):
      x: [H, n_images*STRIDE + 2]  image k's column w at STRIDE*k + 2 + w,
         cols {STRIDE*k, STRIDE*k+1} and the trailing 2 are zeros.
      y: [H, n_images*W]           image k's column w at W*k + w.

    bands[0:4] are the fused HxW band matrices (4-pass path); bands[4] is
    the H-only band (offload path; for mode='box' it is pre-scaled by the
    W tap scale so the box chain needs no multiply).

    mode: None (all 4-pass) | 'box' (taps prop. to [1,3,3,1], 3-add chain)
          | 'sym' (symmetric taps, 5-op chain) | 'gen' (7-op chain).
    """
    if mode == "box" and off_target > TILE:
        off_target = 21          # SEG_FULL: prebox images per full tile
    nc = bacc.Bacc("TRN2", target_bir_lowering=False, debug=False)
    f32 = mybir.dt.float32
    bf16 = mybir.dt.bfloat16

    x_d = nc.dram_tensor("x", [H, n_images * STRIDE + 4], bf16, kind="ExternalInput")
    b_d = nc.dram_tensor("bands", [5, H, H], bf16, kind="ExternalInput")
    y_d = nc.dram_tensor("y", [H, n_images * W], bf16, kind="ExternalOutput")

    tiles = _tiles(n_images)
    segs = _off_plan(tiles, mode, off_target)

    with ExitStack() as ctx:
        tc = ctx.enter_context(tile.TileContext(nc))
        wpool = ctx.enter_context(tc.tile_pool(name="wpool", bufs=1))
        xpool = ctx.enter_context(tc.tile_pool(name="xpool", bufs=xt_bufs))
        opool = ctx.enter_context(tc.tile_pool(name="opool", bufs=5))
        tapool = ctx.enter_context(tc.tile_pool(name="tapool", bufs=3))
        wkpool = ctx.enter_context(tc.tile_pool(name="wkpool", bufs=6))
        tailpool = ctx.enter_context(tc.tile_pool(name="tailpool", bufs=3))
        ppool = ctx.enter_context(tc.tile_pool(name="ppool", bufs=4, space="PSUM"))

        # All DMA via the two HWDGE rings.  Input tiles ride the SP (sync)
        # ring; the band matrices + output tiles ride the ACT (scalar) ring,
        # which is idle early.  GpSimd (SWDGE) stays fully idle.
        xts: dict[int, object] = {}

        def emit_in_dma(ti, pool=None):
            if ti in xts or ti >= len(tiles):
                return
            i0, tn = tiles[ti]
            p = pool if pool is not None else xpool
            xt = p.tile([H, tn * STRIDE + 4], bf16, tag="xt", name="xt")
            nc.sync.dma_start(
                xt, x_d[:, i0 * STRIDE : i0 * STRIDE + tn * STRIDE + 4]
            )
            xts[ti] = xt

        # The last TAILN tiles' inputs are prefetched early into a dedicated
        # pool: the drain compute chains are then never input-gated, which
        # keeps the final output DMAs back-to-back (and the PE busy enough
        # that HAM doesn't re-throttle it mid-drain).
        # Tail-input prefetch (TAILN>0) was tried and regresses: the Sync
        # HWDGE ring is FIFO, so inserting the tail tiles' DMAs anywhere
        # ahead of schedule delays the paced input stream behind them (head
        # insert -> ramp starves, PE idles, HAM half-clocks; mid-run insert
        # -> the next tiles' inputs land late).  Keep 0.
        TAILN = 0
        tail_start = max(0, len(tiles) - TAILN)

        # First x tile DMA is the very first Sync-ring instruction.
        emit_in_dma(0)

        wt = wpool.tile([H, 5 * H], bf16)
        nc.scalar.dma_start(
            wt.rearrange("p (j b) -> p j b", b=H), b_d.rearrange("j a b -> a j b")
        )

        # Warm up the PE HAM clock gate with dummy matmuls on a memset tile
        # (no DMA dependency, so warm-up starts as soon as the engines come
        # out of the framework preamble): real matmuls then hit 2.4 GHz
        # within ~1-2us instead of ~5.
        wsrc = wkpool.tile([H, 256], bf16, tag="wk", name="wk")
        nc.vector.memset(wsrc, 0)
        warm = ppool.tile([H, 512 * QG], f32, tag="pt", name="pt")
        for _ in range(14):
            nc.tensor.matmul(
                warm[:, 0:256], wsrc[:, 0:H], wsrc[:, 0:256], start=True, stop=True
            )

        for ti in range(1, min(3, len(tiles))):
            if ti < tail_start:
                emit_in_dma(ti)
        # Tail inputs go out right here, at the head of the Sync ring's
        # FIFO: the early phase is input-only with spare bandwidth, whereas
        # a mid-run insert would delay the paced input stream behind it.
        for tj in range(tail_start, len(tiles)):
            emit_in_dma(tj, pool=tailpool)

        def tA_view(tA, d, nseg):
            """[p, nseg, W] view of the gap-layout tile shifted by d cols."""
            span = nseg * STRIDE
            if d <= 0:
                sl = tA[:, PAD0 + d : PAD0 + d + span]
                lo = 0
            else:
                sl = tA[:, PAD0 : PAD0 + span]
                lo = d
            return sl.rearrange("p (k c) -> p k c", c=STRIDE)[:, :, lo : lo + W]

        # Per-tile output DMA.  In box mode it is issued from GpSimd (SWDGE,
        # otherwise idle) so the Scalar engine's strict-FIFO evacuation
        # stream is never blocked waiting on Vector's W-conv for a previous
        # tile — that serialization stalled PSUM drain and re-throttled the
        # PE HAM clock.  (Safe here: box-mode Vector only uses 1-port DVE
        # modes, so Q7 descriptor-gen doesn't contend for SBUF ports.)
        out_dma_eng = nc.gpsimd if mode == "box" else nc.scalar

        def emit_4pass(xt, ot, img0, n_img, drain=None):
            """4-pass fused HxW chunks for images [img0, img0+n_img) of the
            tile, reading xt directly (gap layout); evacs on Scalar."""
            nonlocal copy_idx
            gs = _groups(n_img)
            chunks = [gs[s : s + QG] for s in range(0, len(gs), QG)]
            for chunk in chunks:
                nq = len(chunk)
                pt = ppool.tile([H, 512 * nq], f32, tag="pt", name="pt")
                # j-outer order amortizes the 4 stationary (band) loads over
                # the whole chunk; j=2 (d=0) first for the full-width
                # has_written-clearing write.
                for idx, j in enumerate((2, 0, 1, 3)):
                    d = j - PAD0
                    for q, (goff, n) in enumerate(chunk):
                        a = PAD0
                        b = STRIDE * n + PAD0 - (PAD0 if d > 0 else 0)
                        base = (img0 + goff) * STRIDE
                        mm = nc.tensor.matmul(
                            pt[:, 512 * q + a : 512 * q + b],
                            wt[:, H * j : H * (j + 1)],
                            xt[:, base + a + d : base + b + d],
                            start=(idx == 0),
                            stop=(idx == 3),
                        )
                        if q > 0:  # same band as q==0: reuse the stationary
                            mm.ins.ldweights = False
                emit_evac(pt, chunk, ot, img0, drain)

        def emit_prebox(xt, ot, n_img, drain=None):
            """2-pass path for images [0, n_img): Vector pre-convolves the
            raw input with [1,2,1] along W (two flat box-2 adds, 2x DVE
            mode), then PE applies the scaled H band at W-shifts {-2,-1}:
              u1[c] = x[c] + x[c+1]
              u2[c] = u1[c] + u1[c+1]            ( = x *w [1,2,1] )
              psum[:, c] = Bh_s^T (u2[:, c-2] + u2[:, c-1])   ( full blur )
            Vector feeds the INPUT side only — no Scalar->Vector->output
            chain — so evacuation/output never waits on DVE progress."""
            nonlocal copy_idx
            L = n_img * STRIDE + 4
            wk1 = wkpool.tile([H, L - 1], bf16, tag="wk", name="wk")
            wk2 = wkpool.tile([H, L - 2], bf16, tag="wk", name="wk")
            nc.vector.tensor_add(wk1, xt[:, 0 : L - 1], xt[:, 1:L])
            nc.vector.tensor_add(wk2, wk1[:, 0 : L - 2], wk1[:, 1 : L - 1])
            gs = _groups(n_img)
            chunks = [gs[s : s + QG] for s in range(0, len(gs), QG)]
            for chunk in chunks:
                nq = len(chunk)
                pt = ppool.tile([H, 512 * nq], f32, tag="pt", name="pt")
                first = True
                for idx, dd in enumerate((-2, -1)):
                    for q, (goff, n) in enumerate(chunk):
                        a = PAD0
                        b = STRIDE * n + PAD0
                        base = goff * STRIDE
                        mm = nc.tensor.matmul(
                            pt[:, 512 * q + a : 512 * q + b],
                            wt[:, 4 * H : 5 * H],
                            wk2[:, base + a + dd : base + b + dd],
                            start=(idx == 0),
                            stop=(idx == 1),
                        )
                        # all matmuls in this chunk share one stationary
                        # (the scaled H band): only the first loads the PE
                        # array; the rest reuse it (skip LDWEIGHTS).
                        if not first:
                            mm.ins.ldweights = False
                        first = False
                emit_evac(pt, chunk, ot, 0, drain)

        def emit_evac(pt, chunk, ot, img0, drain=None):
            """PSUM -> SBUF evacuation (fp32 -> bf16, gap strip).  One
            strided copy per chunk when the chunk is uniform (all groups
            GROUP-sized); per-group copies otherwise (ragged tail).

            drain=(tile_i0,) streams this chunk's images straight out after
            the copy (small per-chunk DMAs, alternating rings): in the
            kernel's final tiles this overlaps the last evacuations with
            the output drain instead of waiting for the whole tile."""
            nonlocal copy_idx, dma_alt
            nq = len(chunk)
            uniform = all(n == GROUP for _, n in chunk)
            if mode is not None:
                # All evacs on Scalar: a Vector-assigned evac would sit in
                # Vector's FIFO ahead of the next tile's input-side work.
                eng = (nc.scalar, nc.scalar)
            else:
                eng = (nc.vector, nc.scalar)
            if uniform:
                psrc = (
                    pt.rearrange("p (q c) -> p q c", c=512)[:, :, : GROUP * STRIDE]
                    .rearrange("p q (k c) -> p q k c", c=STRIDE)[
                        :, :, :, PAD0 : PAD0 + W
                    ]
                )
                odst = ot[
                    :,
                    (img0 + chunk[0][0]) * W : (img0 + chunk[-1][0] + GROUP) * W,
                ].rearrange("p (q k c) -> p q k c", q=nq, c=W)
                e = eng[copy_idx % 2]
                if e is nc.vector:
                    e.tensor_copy(odst, psrc)
                else:
                    e.copy(odst, psrc)
                copy_idx += 1
            else:
                for q, (goff, n) in enumerate(chunk):
                    psrc = pt[:, 512 * q : 512 * q + STRIDE * n].rearrange(
                        "p (k c) -> p k c", c=STRIDE
                    )[:, :, PAD0 : PAD0 + W]
                    odst = ot[
                        :, (img0 + goff) * W : (img0 + goff + n) * W
                    ].rearrange("p (k c) -> p k c", c=W)
                    e = eng[copy_idx % 2]
                    if e is nc.vector:
                        e.tensor_copy(odst, psrc)
                    else:
                        e.copy(odst, psrc)
                    copy_idx += 1
            if drain is not None:
                tile_i0 = drain[0]
                lo = img0 + chunk[0][0]
                hi = img0 + chunk[-1][0] + chunk[-1][1]
                e_dma = nc.scalar if dma_alt % 2 == 0 else nc.gpsimd
                dma_alt += 1
                e_dma.dma_start(
                    y_d[:, (tile_i0 + lo) * W : (tile_i0 + hi) * W],
                    ot[:, lo * W : hi * W],
                )

        copy_idx = 0
        dma_alt = 0
        n_t = len(tiles)
        for ti, (i0, tn) in enumerate(tiles):
            if ti + 3 < tail_start:
                emit_in_dma(ti + 3)
            xt = xts.pop(ti)

            seg = segs[ti]
            n_norm = tn - seg
            ot = opool.tile([H, tn * W], bf16, tag="ot", name="ot")

            if mode == "box":
                # prebox portion is images [0, seg); 4-pass the rest.  The
                # u-chain goes first (Vector, input side); the 4-pass chunks
                # (independent of it) keep PE busy while it runs.
                if n_norm:
                    emit_4pass(xt, ot, seg, n_norm)
                if seg:
                    emit_prebox(xt, ot, seg)
                # Drain tiles alternate output rings (SWDGE + ACT HWDGE):
                # two queues keep the SDMA engines fed in the output-only
                # tail, and Scalar is idle by then anyway.
                if ti >= n_t - 6:
                    eng_out = nc.scalar if (n_t - ti) % 2 == 0 else nc.gpsimd
                else:
                    eng_out = nc.gpsimd
                eng_out.dma_start(y_d[:, i0 * W : (i0 + tn) * W], ot)
                continue

            # --- non-box modes: original structure (offload seg last, via
            # tA intermediate; 4-pass first) ---
            if seg:
                segc0 = n_norm * STRIDE
                span = seg * STRIDE + 2
                tA = tapool.tile([H, span], bf16, tag="ta", name="ta")
                c0 = 0
                while c0 < span:
                    cw = min(512 * QG, span - c0)
                    pt = ppool.tile([H, 512 * QG], f32, tag="pt", name="pt")
                    s = 0
                    while s < cw:
                        w_ = min(512, cw - s)
                        nc.tensor.matmul(
                            pt[:, s : s + w_],
                            wt[:, 4 * H : 5 * H],
                            xt[:, segc0 + c0 + s : segc0 + c0 + s + w_],
                            start=True,
                            stop=True,
                        )
                        s += w_
                    nc.scalar.copy(tA[:, c0 : c0 + cw], pt[:, 0:cw])
                    c0 += cw

                e = nc.vector
                otv = ot[:, n_norm * W : tn * W].rearrange("p (k c) -> p k c", c=W)
                if True:
                    v = [tA_view(tA, d, seg) for d in (-2, -1, 0, 1)]
                    wk1 = wkpool.tile([H, seg * W], bf16, tag="wk", name="wk")
                    wk2 = wkpool.tile([H, seg * W], bf16, tag="wk", name="wk")
                    wk3 = wkpool.tile([H, seg * W], bf16, tag="wk", name="wk")
                    w1v = wk1.rearrange("p (k c) -> p k c", c=W)
                    w2v = wk2.rearrange("p (k c) -> p k c", c=W)
                    w3v = wk3.rearrange("p (k c) -> p k c", c=W)
                    if mode == "sym":
                        e.tensor_add(w1v, v[0], v[3])
                        e.tensor_add(w2v, v[1], v[2])
                        e.tensor_scalar_mul(w3v, w1v, taps[0])
                        e.tensor_scalar_mul(w1v, w2v, taps[1])
                        e.tensor_add(otv, w3v, w1v)
                    else:
                        e.tensor_scalar_mul(w1v, v[0], taps[0])
                        e.tensor_scalar_mul(w2v, v[1], taps[1])
                        e.tensor_add(w3v, w1v, w2v)
                        e.tensor_scalar_mul(w1v, v[2], taps[2])
                        e.tensor_add(w2v, w3v, w1v)
                        e.tensor_scalar_mul(w1v, v[3], taps[3])
                        e.tensor_add(otv, w2v, w1v)

            gs = _groups(n_norm)
            chunks = [gs[s : s + QG] for s in range(0, len(gs), QG)]

            for chunk in chunks:
                nq = len(chunk)
                pt = ppool.tile([H, 512 * nq], f32, tag="pt", name="pt")
                # j-outer order amortizes the 4 stationary (band) loads over
                # the whole chunk; j=2 (d=0) first for the full-width
                # has_written-clearing write.
                for idx, j in enumerate((2, 0, 1, 3)):
                    d = j - PAD0
                    for q, (goff, n) in enumerate(chunk):
                        a = PAD0
                        b = STRIDE * n + PAD0 - (PAD0 if d > 0 else 0)
                        base = goff * STRIDE
                        nc.tensor.matmul(
                            pt[:, 512 * q + a : 512 * q + b],
                            wt[:, H * j : H * (j + 1)],
                            xt[:, base + a + d : base + b + d],
                            start=(idx == 0),
                            stop=(idx == 3),
                        )

                # PSUM -> SBUF evacuation (fp32 -> bf16).  One strided copy
                # per chunk when the chunk is uniform (all groups GROUP-sized);
                # per-group copies otherwise (ragged tail).  Scalar carries
                # the evacuations; Vector relieves it on every 8th chunk.
                uniform = all(n == GROUP for _, n in chunk)
                if mode is not None:
                    e_pick = nc.vector if copy_idx % 8 == 7 else nc.scalar
                    eng = (e_pick, e_pick)
                else:
                    eng = (nc.vector, nc.scalar)
                if uniform:
                    psrc = (
                        pt.rearrange("p (q c) -> p q c", c=512)[
                            :, :, : GROUP * STRIDE
                        ]
                        .rearrange("p q (k c) -> p q k c", c=STRIDE)[
                            :, :, :, PAD0 : PAD0 + W
                        ]
                    )
                    odst = ot[
                        :, chunk[0][0] * W : (chunk[-1][0] + GROUP) * W
                    ].rearrange("p (q k c) -> p q k c", q=nq, c=W)
                    e = eng[copy_idx % 2]
                    if e is nc.vector:
                        e.tensor_copy(odst, psrc)
                    else:
                        e.copy(odst, psrc)
                    copy_idx += 1
                else:
                    for q, (goff, n) in enumerate(chunk):
                        psrc = pt[:, 512 * q : 512 * q + STRIDE * n].rearrange(
                            "p (k c) -> p k c", c=STRIDE
                        )[:, :, PAD0 : PAD0 + W]
                        odst = ot[:, goff * W : (goff + n) * W].rearrange(
                            "p (k c) -> p k c", c=W
                        )
                        e = eng[copy_idx % 2]
                        if e is nc.vector:
                            e.tensor_copy(odst, psrc)
                        else:
                            e.copy(odst, psrc)
                        copy_idx += 1

            out_dma_eng.dma_start(y_d[:, i0 * W : (i0 + tn) * W], ot)

    nc.compile()
    return nc


def _get_program(n_images: int, mode=None, taps=None, off_target=282):
    key = (n_images, mode, taps, off_target)
    if key not in _PROGRAM_CACHE:
        _PROGRAM_CACHE[key] = build_program(
            n_images, mode=mode, taps=taps, off_target=off_target
        )
    return _PROGRAM_CACHE[key]


def _separable(kern: np.ndarray):
    """Return (bands5_f32, mode, taps).

    mode None: not rank-1 (all-PE 4-pass).  mode 'box': W taps proportional
    to [1,3,3,1]; the scale is folded into bands5[4] and taps is None.
    mode 'sym'/'gen': rank-1 with symmetric/general taps (5/7-op W-conv).
    """
    K = np.asarray(kern, dtype=np.float64)
    bands5 = np.zeros((5, H, H), dtype=np.float32)
    bands5[0:4] = _band_matrices(kern)
    U, S, Vt = np.linalg.svd(K)
    if S[1] > 1e-6 * max(S[0], 1e-30):
        return bands5, None, None
    a = U[:, 0] * np.sqrt(S[0])
    b = Vt[0, :] * np.sqrt(S[0])
    af = a[::-1]  # flipped H factor
    bfl = b[::-1]  # flipped W factor -> the 4 free-dim taps

    ref = np.array([1.0, 3.0, 3.0, 1.0])
    s = bfl[0]
    if abs(s) > 1e-30 and np.allclose(bfl, s * ref, rtol=1e-5, atol=0):
        mode, taps, af_eff = "box", None, af * s
    else:
        mode = "sym" if abs(bfl[0] - bfl[3]) <= 1e-7 * max(
            abs(bfl[0]), abs(bfl[3])
        ) and abs(bfl[1] - bfl[2]) <= 1e-7 * max(abs(bfl[1]), abs(bfl[2])) else "gen"
        taps, af_eff = tuple(float(np.float32(v)) for v in bfl), af

    ho = np.arange(H)
    Bh = np.zeros((H, H), dtype=np.float64)
    for i in range(4):
        hi = ho + (i - PAD0)
        m = (hi >= 0) & (hi < H)
        Bh[hi[m], ho[m]] = af_eff[i]
    bands5[4] = Bh.astype(np.float32)
    return bands5, mode, taps


def _pack_input(xc_bf16: np.ndarray) -> np.ndarray:
    """[n, H, W] bf16 -> [H, n*STRIDE + 4] bf16 gap layout."""
    n = xc_bf16.shape[0]
    arr = np.zeros((H, n * STRIDE + 4), dtype=BF16)
    v = np.lib.stride_tricks.as_strided(
        arr,
        shape=(H, n, STRIDE),
        strides=(arr.strides[0], STRIDE * arr.itemsize, arr.itemsize),
    )
    v[:, :, PAD0:] = xc_bf16.transpose(1, 0, 2)
    return arr


def kernel(x: np.ndarray, kernel: np.ndarray, _trace: bool = False):
    x = np.ascontiguousarray(x, dtype=np.float32)
    assert x.shape == (B, C, H, W), x.shape
    bands5, mode, taps = _separable(kernel)
    bands_bf = bands5.astype(BF16)

    n_total = B * C
    n_per_core = n_total // N_CORES
    xb = x.reshape(n_total, H, W).astype(BF16)

    nc = _get_program(n_per_core, mode, taps)
    in_maps = [
        {
            "x": _pack_input(xb[c * n_per_core : (c + 1) * n_per_core]),
            "bands": bands_bf,
        }
        for c in range(N_CORES)
    ]
    res = bass_utils.run_bass_kernel_spmd(
        nc, in_maps, core_ids=list(range(N_CORES)), trace=_trace
    )
    y = np.empty((n_total, H, W), dtype=np.float32)
    for c, r in enumerate(res.results):
        yc = np.asarray(r["y"]).reshape(H, n_per_core, W)
        y[c * n_per_core : (c + 1) * n_per_core] = yc.transpose(1, 0, 2).astype(
            np.float32
        )
    y = y.reshape(B, C, H, W)
    if _trace:
        return y, res
    return y
